# revision 2
# baseline (speedup 1.0000x reference)
"""Bahdanau additive attention on 8 Trainium2 NeuronCores.

Math (per batch element b):
    ep = enc @ W1 + b1                      # [S, U]
    dp = dec @ W2 + b2                      # [T, U]
    score[t,s,:] = tanh(ep[s,:] + dp[t,:]) + (1-mask[s])*NEG
    logits[t,s] = score[t,s,:] @ Wv + bv    # [T, S]
    weights = softmax_s(logits)             # [T, S]
    context = weights @ enc                 # [T, D]

Sharding: data-parallel over batch B=8, one batch element per core.

Per-core layout strategy:
  - epT [u-part(4x128), s-free], dpT [u-part, t-free] computed on PE from
    transposed enc/dec (PE transposes) with W1/W2 natural-layout stationaries.
  - X[t-block] = epT + dpT[:,t] via DVE tensor_scalar per-partition adds
    (u on partitions), batched TG target steps per tile.
  - tanh on the scalar engine in large [128, TG*256] instructions, bf16 out.
  - Reduction over u via PE: stationary = tanh tile slice [128u, 128s],
    moving = Wv column [128u, 1] -> logitsT column [128s, 1] accumulated in
    PSUM over the 4 u-chunks.  PSUM bank is opened by one rank-1 "opener"
    matmul qneg (x) ones (start=True, writes every byte) which also applies
    the mask additive term (1-m[s])*NEG*sum(Wv); all column matmuls then
    accumulate per-byte (start=False), so their relative order is free.
  - softmax without max-subtraction (logits are O(1); masked entries arrive
    as -1e9*sum(Wv) pre-exp, matching the reference's pre-Dense(1) mask add):
    exp on ACT (PSUM->SBUF), column sums via PE with expT stationary,
    reciprocal on DVE, and per-partition (t) rescale fused into the
    PSUM->SBUF copies of both outputs.
  - context = (expT)^T @ enc on PE, scaled by recip(sums) per partition.
  - attention weights output = transpose(expT) scaled by recip(sums).

bv is omitted: softmax over s is invariant to per-(t) constants (and bv = 0
in the problem spec).
"""

import numpy as np

B, S, T, D, U = 8, 256, 128, 512, 512
NEG = -1.0e9
TG = 16  # target steps per X/Y tile
N_CORES = 8

_CACHED_NC = None


def _build_nc():
    from contextlib import ExitStack

    import concourse.bass as bass
    import concourse.tile as tile
    from concourse import bacc, mybir
    from concourse.masks import make_identity

    f32 = mybir.dt.float32
    bf16 = mybir.dt.bfloat16
    u8 = mybir.dt.uint8
    Act = mybir.ActivationFunctionType
    Alu = mybir.AluOpType

    nc = bacc.Bacc(
        "TRN2",
        target_bir_lowering=False,
        debug=False,
        enable_asserts=False,
        num_devices=N_CORES,
    )

    enc_d = nc.dram_tensor("enc", [S, D], f32, kind="ExternalInput")
    dec_d = nc.dram_tensor("dec", [T, D], f32, kind="ExternalInput")
    mask_d = nc.dram_tensor("mask", [S], u8, kind="ExternalInput")
    w1_d = nc.dram_tensor("w1", [D, U], f32, kind="ExternalInput")
    b1_d = nc.dram_tensor("b1", [U], f32, kind="ExternalInput")
    w2_d = nc.dram_tensor("w2", [D, U], f32, kind="ExternalInput")
    b2_d = nc.dram_tensor("b2", [U], f32, kind="ExternalInput")
    wv_d = nc.dram_tensor("wv", [U], f32, kind="ExternalInput")
    ctx_d = nc.dram_tensor("ctx_out", [T, D], f32, kind="ExternalOutput")
    wout_d = nc.dram_tensor("w_out", [T, S], f32, kind="ExternalOutput")

    UC = U // 128  # 4 u chunks
    DC = D // 128  # 4 d chunks
    SB = S // 128  # 2 s blocks
    NTG = T // TG

    with tile.TileContext(nc) as tc, ExitStack() as ctx:
        singles = ctx.enter_context(tc.tile_pool(name="singles", bufs=1))

        # ---- constant / input loads -------------------------------------
        identity = singles.tile([128, 128], f32, tag="identity")
        make_identity(nc, identity)

        enc_sb = singles.tile([128, SB, D], f32, tag="enc_sb")  # [s, sb, d]
        nc.gpsimd.dma_start(enc_sb[:], enc_d.ap().rearrange("(sb p) d -> p sb d", p=128))
        dec_sb = singles.tile([128, D], f32, tag="dec_sb")  # [t, d]
        nc.gpsimd.dma_start(dec_sb[:], dec_d.ap())
        w1_sb = singles.tile([128, DC, U], f32, tag="w1_sb")  # [d, dc, u]
        nc.gpsimd.dma_start(w1_sb[:], w1_d.ap().rearrange("(c p) u -> p c u", p=128))
        w2_sb = singles.tile([128, DC, U], f32, tag="w2_sb")
        nc.gpsimd.dma_start(w2_sb[:], w2_d.ap().rearrange("(c p) u -> p c u", p=128))
        wv_col = singles.tile([128, UC], f32, tag="wv_col")  # [u, uc]
        nc.gpsimd.dma_start(wv_col[:], wv_d.ap().rearrange("(c p) -> p c", p=128))
        b1_col = singles.tile([128, UC], f32, tag="b1_col")
        nc.gpsimd.dma_start(b1_col[:], b1_d.ap().rearrange("(c p) -> p c", p=128))
        b2_col = singles.tile([128, UC], f32, tag="b2_col")
        nc.gpsimd.dma_start(b2_col[:], b2_d.ap().rearrange("(c p) -> p c", p=128))
        mask_row = singles.tile([1, S], u8, tag="mask_row")
        nc.gpsimd.dma_start(mask_row[:], mask_d.ap()[None, :])

        wv_bf = singles.tile([128, UC], bf16, tag="wv_bf")
        nc.vector.tensor_copy(wv_bf[:], wv_col[:])

        ones_col = singles.tile([128, 1], f32, tag="ones_col")
        nc.vector.memset(ones_col[:], 1.0)
        ones_row_bf = singles.tile([1, 128], bf16, tag="ones_row_bf")
        nc.vector.memset(ones_row_bf[:], 1.0)

        # ---- stage A: transposes, projections, mask row -----------------
        with tc.tile_pool(name="psA", bufs=2, space="PSUM") as psA:
            # sum(Wv): [128,UC] --free-reduce--> [128,1] --PE--> [1,1]
            wv_psum = singles.tile([128, 1], f32, tag="wv_psum")
            nc.vector.tensor_reduce(
                out=wv_psum[:], in_=wv_col[:], axis=mybir.AxisListType.X, op=Alu.add
            )
            sw_ps = psA.tile([1, 1], f32, tag="sw")
            nc.tensor.matmul(sw_ps[:], lhsT=wv_psum[:], rhs=ones_col[:], start=True, stop=True)
            sumwv = singles.tile([1, 1], f32, tag="sumwv")
            nc.vector.tensor_copy(sumwv[:], sw_ps[:])

            # qneg_row[s] = (1 - mask[s]) * NEG * sum(Wv), as bf16 row
            mask_f = singles.tile([1, S], f32, tag="mask_f")
            nc.vector.tensor_copy(mask_f[:], mask_row[:])
            qneg_row = singles.tile([1, S], f32, tag="qneg_row")
            nc.vector.tensor_scalar(
                out=qneg_row[:], in0=mask_f[:], scalar1=-NEG, scalar2=NEG,
                op0=Alu.mult, op1=Alu.add,
            )
            nc.vector.tensor_scalar_mul(qneg_row[:], in0=qneg_row[:], scalar1=sumwv[:])
            qneg_bf = singles.tile([1, S], bf16, tag="qneg_bf")
            nc.vector.tensor_copy(qneg_bf[:], qneg_row[:])

            # encT [d, dc, s], decT [d, dc, t] via PE transposes
            encT = singles.tile([128, DC, S], f32, tag="encT")
            for dc in range(DC):
                for sb in range(SB):
                    tp = psA.tile([128, 128], f32, tag="tp")
                    nc.tensor.transpose(tp[:], enc_sb[:, sb, dc * 128:(dc + 1) * 128], identity[:])
                    nc.vector.tensor_copy(encT[:, dc, sb * 128:(sb + 1) * 128], tp[:])
            decT = singles.tile([128, DC, T], f32, tag="decT")
            for dc in range(DC):
                tp = psA.tile([128, 128], f32, tag="tp")
                nc.tensor.transpose(tp[:], dec_sb[:, dc * 128:(dc + 1) * 128], identity[:])
                nc.vector.tensor_copy(decT[:, dc, :], tp[:])

            # epT [u, uc, s] = W1^T encT + b1 ; dpT [u, uc, t] = W2^T decT + b2
            epT = singles.tile([128, UC, S], f32, tag="epT")
            for uc in range(UC):
                pe = psA.tile([128, S], f32, tag="proj")
                for dc in range(DC):
                    nc.tensor.matmul(
                        pe[:], lhsT=w1_sb[:, dc, uc * 128:(uc + 1) * 128],
                        rhs=encT[:, dc, :], start=(dc == 0), stop=(dc == DC - 1),
                    )
                nc.vector.tensor_scalar(
                    out=epT[:, uc, :], in0=pe[:], scalar1=b1_col[:, uc:uc + 1],
                    scalar2=None, op0=Alu.add,
                )
            dpT = singles.tile([128, UC, T], f32, tag="dpT")
            for uc in range(UC):
                pd = psA.tile([128, S], f32, tag="proj")
                for dc in range(DC):
                    nc.tensor.matmul(
                        pd[:, 0:T], lhsT=w2_sb[:, dc, uc * 128:(uc + 1) * 128],
                        rhs=decT[:, dc, :], start=(dc == 0), stop=(dc == DC - 1),
                    )
                nc.vector.tensor_scalar(
                    out=dpT[:, uc, :], in0=pd[:, 0:T], scalar1=b2_col[:, uc:uc + 1],
                    scalar2=None, op0=Alu.add,
                )

        # ---- stage B: tanh + weighted u-reduction -> logitsT psum -------
        lg_pool = ctx.enter_context(tc.tile_pool(name="lg", bufs=1, space="PSUM"))
        lgT = [
            lg_pool.tile([128, T], f32, tag=f"lg{sb}", name=f"lg{sb}")
            for sb in range(SB)
        ]

        # opener: logitsT[s, t] = qneg[s] (applies mask term, opens the bank)
        for sb in range(SB):
            nc.tensor.matmul(
                lgT[sb][:],
                lhsT=qneg_bf[:, sb * 128:(sb + 1) * 128],
                rhs=ones_row_bf[:],
                start=True, stop=False,
            )

        xpool = ctx.enter_context(tc.tile_pool(name="xp", bufs=3))
        ypool = ctx.enter_context(tc.tile_pool(name="yp", bufs=2))

        for tg in range(NTG):
            ys = []
            for uc in range(UC):
                x = xpool.tile([128, TG * S], f32, tag="x")
                for tl in range(TG):
                    t = tg * TG + tl
                    nc.vector.tensor_scalar(
                        out=x[:, tl * S:(tl + 1) * S], in0=epT[:, uc, :],
                        scalar1=dpT[:, uc, t:t + 1], scalar2=None, op0=Alu.add,
                    )
                y = ypool.tile([128, TG * S], bf16, tag=f"y{uc}")
                nc.scalar.activation(y[:], x[:], Act.Tanh)
                ys.append(y)
            for tl in range(TG):
                t = tg * TG + tl
                for sb in range(SB):
                    for uc in range(UC):
                        last = (tg == NTG - 1) and (tl == TG - 1) and (uc == UC - 1)
                        nc.tensor.matmul(
                            lgT[sb][:, t:t + 1],
                            lhsT=ys[uc][:, tl * S + sb * 128: tl * S + (sb + 1) * 128],
                            rhs=wv_bf[:, uc:uc + 1],
                            start=False, stop=last,
                        )

        # ---- stage C: softmax + outputs ---------------------------------
        with tc.tile_pool(name="psC", bufs=1, space="PSUM") as psC:
            expT = []
            for sb in range(SB):
                e = singles.tile([128, T], f32, tag=f"expT{sb}")
                nc.scalar.activation(e[:], lgT[sb][:], Act.Exp)
                expT.append(e)

            sums_ps = psC.tile([128, 1], f32, tag="sums")
            for sb in range(SB):
                nc.tensor.matmul(
                    sums_ps[:], lhsT=expT[sb][:], rhs=ones_col[:],
                    start=(sb == 0), stop=(sb == SB - 1),
                )
            recip = singles.tile([128, 1], f32, tag="recip")
            nc.vector.reciprocal(recip[:], sums_ps[:])

            ctx_ps = psC.tile([128, D], f32, tag="ctxp")
            for sb in range(SB):
                nc.tensor.matmul(
                    ctx_ps[:], lhsT=expT[sb][:], rhs=enc_sb[:, sb, :],
                    start=(sb == 0), stop=(sb == SB - 1),
                )
            ctx_sb = singles.tile([128, D], f32, tag="ctx_sb")
            nc.vector.tensor_scalar_mul(ctx_sb[:], in0=ctx_ps[:], scalar1=recip[:])
            nc.gpsimd.dma_start(ctx_d.ap(), ctx_sb[:])

            w_sb = singles.tile([128, S], f32, tag="w_sb")
            for sb in range(SB):
                wps = psC.tile([128, 128], f32, tag="wps")
                nc.tensor.transpose(wps[:], expT[sb][:], identity[:])
                nc.vector.tensor_scalar_mul(
                    w_sb[:, sb * 128:(sb + 1) * 128], in0=wps[:], scalar1=recip[:]
                )
            nc.gpsimd.dma_start(wout_d.ap(), w_sb[:])

    nc.compile()
    return nc


def _get_nc():
    global _CACHED_NC
    if _CACHED_NC is None:
        _CACHED_NC = _build_nc()
    return _CACHED_NC


def kernel(encoder_output, decoder_output, mask, W1, b1, W2, b2, Wv, bv,
           _trace=False):
    from concourse.bass_utils import run_bass_kernel_spmd

    nc = _get_nc()
    W1 = np.ascontiguousarray(np.asarray(W1, dtype=np.float32))
    W2 = np.ascontiguousarray(np.asarray(W2, dtype=np.float32))
    b1 = np.ascontiguousarray(np.asarray(b1, dtype=np.float32))
    b2 = np.ascontiguousarray(np.asarray(b2, dtype=np.float32))
    wv = np.ascontiguousarray(np.asarray(Wv, dtype=np.float32).reshape(U))
    in_maps = []
    for b in range(B):
        in_maps.append({
            "enc": np.ascontiguousarray(np.asarray(encoder_output[b], dtype=np.float32)),
            "dec": np.ascontiguousarray(np.asarray(decoder_output[b], dtype=np.float32)),
            "mask": np.ascontiguousarray(np.asarray(mask[b]).astype(np.uint8)),
            "w1": W1, "b1": b1, "w2": W2, "b2": b2, "wv": wv,
        })
    res = run_bass_kernel_spmd(nc, in_maps, core_ids=list(range(N_CORES)),
                               trace=_trace)
    context = np.stack([res.results[b]["ctx_out"] for b in range(B)])
    weights = np.stack([res.results[b]["w_out"] for b in range(B)])[..., None]
    if _trace:
        kernel._last_result = res
    return context.astype(np.float32), weights.astype(np.float32)
